# revision 2
# baseline (speedup 1.0000x reference)
"""DiT attention (B=2, S=2048, DIM=1024, H=16, D=64) on 8 TRN2 NeuronCores, v2.

Sharding: data-parallel over B (2) x tensor-parallel over head groups (4);
each core owns (one batch, 4 heads = 2 head-pairs). Host sums the 4 partial
out-projections per batch and adds out_b.

v2 changes vs baseline (259us):
  - PE array packing: QK^T runs 2 heads concurrently via 64x128 row tiling
    (tile_position (0,0)/(64,0)); AV runs 2 heads concurrently via 128x64
    column tiling; softmax denominators via 4-way 128x32 column tiling with
    an all-ones stationary (replaces the padded-V ones-column trick, so AV
    does no wasted columns).
  - softmax exp split across two engines: ScalarE table exp and VectorE
    Schraudolph exp (one tensor_scalar fma -> int16 bits bitcast to bf16;
    ~1.8% rms weight error, zero-mean; output error ~0.1% since attention
    is diffuse).
  - projections restructured for stationary reuse (weights stay loaded for
    4 matmuls) and drained on ScalarE (Identity+bias) to keep DVE free.
  - warmup matmuls during the initial DMA wait so HAM reaches K=8/8 before
    the real work starts; hT DMA'd in half-row chunks so the first
    projection matmuls start at ~1.5us.
"""

import numpy as np
import ml_dtypes

import concourse.bacc as bacc
import concourse.bass as bass
import concourse.mybir as mybir
import concourse.tile as tile
from concourse.bass_utils import run_bass_kernel_spmd

B, S, DIM, H, D = 2, 2048, 1024, 16, 64
NCORES = 8
GROUPS = 4     # head groups (tensor parallel)
HPG = 4        # heads per group -> 2 pairs
E = HPG * D    # 256 e-channels per core per projection
P = 128
SC = 512       # query-chunk width
NKT = S // P   # 16 key tiles
NQC = S // SC  # 4 query chunks
KT8 = DIM // P  # 8 contraction tiles
BF = mybir.dt.bfloat16
F32 = mybir.dt.float32
I16 = mybir.dt.int16

_BF_NP = ml_dtypes.bfloat16

# Schraudolph exp constants for bf16-bit output: bits = round(s*SCH_A + SCH_B)
# approximates exp(0.125*s).  SCH_A = 0.125 * 128/ln2;  C=7.35 zero-means the
# mantissa-linearization error.
SCH_A = float(16.0 / np.log(2.0))
SCH_B = float(127 * 128 - 7.35)


def _build_nc():
    nc = bacc.Bacc(None, target_bir_lowering=False)

    hT_d = nc.declare_dram_parameter("hT", [DIM, S], BF, isOutput=False)
    wqkvT_d = nc.declare_dram_parameter("wqkvT", [DIM, 3 * E], BF, isOutput=False)
    qkvb_d = nc.declare_dram_parameter("qkvb", [1, 3 * E], BF, isOutput=False)
    qkvbc_d = nc.declare_dram_parameter("qkvb_col", [2 * E, 1], F32, isOutput=False)
    woutT_d = nc.declare_dram_parameter("woutT", [E, DIM], BF, isOutput=False)
    cos_d = nc.declare_dram_parameter("cos_t", [D, S], BF, isOutput=False)
    sin_d = nc.declare_dram_parameter("sin_t", [D, S], BF, isOutput=False)
    perm_d = nc.declare_dram_parameter("perm", [D, D], BF, isOutput=False)
    y_d = nc.declare_dram_parameter("y", [S, DIM], BF, isOutput=True)

    hT_t = hT_d.ap().rearrange("(t p) s -> t p s", p=P)        # [8,128,S]
    wqkvT_t = wqkvT_d.ap().rearrange("(t p) e -> t p e", p=P)  # [8,128,768]
    woutT_t = woutT_d.ap().rearrange("(t p) o -> t p o", p=P)  # [2,128,DIM]

    with tile.TileContext(nc) as tc:
        import contextlib
        with contextlib.ExitStack() as ctx:
            consts = ctx.enter_context(tc.tile_pool(name="consts", bufs=1))
            work = ctx.enter_context(tc.tile_pool(name="work", bufs=2))
            dram = ctx.enter_context(tc.tile_pool(name="dram", bufs=2, space="DRAM"))
            psum = ctx.enter_context(tc.tile_pool(name="psum", bufs=2, space="PSUM"))

            # ---- persistent SBUF ----
            hT_sb = consts.tile([P, KT8, S], BF, name="hT_sb")
            wqkvT_sb = consts.tile([P, KT8, 3 * E], BF, name="wqkvT_sb")
            qkvb_sb = consts.tile([1, 3 * E], BF, name="qkvb_sb")
            qkvbc_sb = consts.tile([P, 4, 1], F32, name="qkvbc_sb")
            woutT_sb = consts.tile([P, E // P, DIM], BF, name="woutT_sb")
            cos_sb = consts.tile([D, S], BF, name="cos_sb")
            sin_sb = consts.tile([D, S], BF, name="sin_sb")
            perm_sb = consts.tile([D, D], BF, name="perm_sb")
            ones1 = consts.tile([1, P], BF, name="ones1")
            ones32 = consts.tile([P, 32], BF, name="ones32")
            warm_sb = consts.tile([P, 256], BF, name="warm_sb")
            V_sb = consts.tile([P, NKT, E], BF, name="V_sb")      # pair-packed, no pad
            QT_sb = consts.tile([P, 2, S], BF, name="QT_sb")      # slot=pair, rows 0-63/64-127
            KT_sb = consts.tile([P, 2, S], BF, name="KT_sb")
            OT_sb = consts.tile([P, 2, S], BF, name="OT_sb")
            q0r = consts.tile([D, S], BF, name="q0r")
            k0r = consts.tile([D, S], BF, name="k0r")
            qtmp = consts.tile([D, S], BF, name="qtmp")
            ktmp = consts.tile([D, S], BF, name="ktmp")

            # ---- warmup (keeps HAM busy during initial DMA) ----
            nc.vector.memset(warm_sb[:, :], 0.0)
            warm_ps = psum.tile([P, SC], F32, name="warm_ps", tag="s_ps", bufs=3)
            for i in range(26):
                nc.tensor.matmul(out=warm_ps[:, 0:256], lhsT=warm_sb[:, 0:P],
                                 rhs=warm_sb[:, :], start=True, stop=True)

            # ---- loads: biases first, then per-kt interleaved weight+hT
            # half0 chunks so the first projection groups unlock ASAP ----
            nc.sync.dma_start(out=qkvb_sb[:, :], in_=qkvb_d.ap())
            for t in range(4):
                nc.sync.dma_start(out=qkvbc_sb[:, t, :],
                                  in_=qkvbc_d.ap()[t * P:(t + 1) * P, :])
            h0 = slice(0, 2 * SC)
            h1 = slice(2 * SC, 4 * SC)
            for t in range(KT8):
                nc.sync.dma_start(out=wqkvT_sb[:, t, :], in_=wqkvT_t[t])
                nc.sync.dma_start(out=hT_sb[:, t, h0], in_=hT_t[t][:, h0])
            for t in range(KT8):
                nc.sync.dma_start(out=hT_sb[:, t, h1], in_=hT_t[t][:, h1])
            nc.sync.dma_start(out=cos_sb[:, :], in_=cos_d.ap())
            nc.sync.dma_start(out=sin_sb[:, :], in_=sin_d.ap())
            nc.sync.dma_start(out=perm_sb[:, :], in_=perm_d.ap())
            for t in range(E // P):
                nc.sync.dma_start(out=woutT_sb[:, t, :], in_=woutT_t[t])
            nc.vector.memset(ones1[:, :], 1.0)
            nc.vector.memset(ones32[:, :], 1.0)

            # ---- Q^T / K^T projections: [e, s], stationary w reused x2 ----
            # et=0 groups first so RoPE (which only needs slot 0) overlaps
            # the et=1 matmuls; V projection is deferred into the attention
            # pipeline as filler work.
            def emit_qk_proj(which, et, half):
                dst = QT_sb if which == 0 else KT_sb
                ecols = slice(which * E + et * P, which * E + (et + 1) * P)
                pj = psum.tile([P, 2, SC], F32, name="pj", tag="s_ps", bufs=3)
                for kt in range(KT8):
                    for j in range(2):
                        s_sl = slice((2 * half + j) * SC,
                                     (2 * half + j + 1) * SC)
                        nc.tensor.matmul(
                            out=pj[:, j, :],
                            lhsT=wqkvT_sb[:, kt, ecols],
                            rhs=hT_sb[:, kt, s_sl],
                            start=(kt == 0), stop=(kt == KT8 - 1))
                for j in range(2):
                    s_sl = slice((2 * half + j) * SC,
                                 (2 * half + j + 1) * SC)
                    nc.scalar.activation(
                        out=dst[:, et, s_sl], in_=pj[:, j, :],
                        func=mybir.ActivationFunctionType.Identity,
                        bias=qkvbc_sb[:, which * 2 + et, :])

            # half0 groups (DMA-paced), then half1; et=0 before et=1 inside a
            # half so RoPE's DVE muls can start as early as possible.
            for half in range(2):
                for et in range(2):
                    for which in range(2):
                        emit_qk_proj(which, et, half)
                if half == 0:
                    continue
                # RoPE DVE muls (emitted after et0 of half1... both halves done)
            nc.vector.tensor_mul(out=qtmp[:, :], in0=QT_sb[0:D, 0, :], in1=cos_sb[:, :])
            nc.vector.tensor_mul(out=ktmp[:, :], in0=KT_sb[0:D, 0, :], in1=cos_sb[:, :])

            for src, dst, tmp in ((QT_sb, q0r, qtmp), (KT_sb, k0r, ktmp)):
                for scn in range(NQC):
                    s_sl = slice(scn * SC, (scn + 1) * SC)
                    sw_ps = psum.tile([D, SC], F32, name="sw_ps", tag="s_ps", bufs=3)
                    nc.tensor.matmul(
                        out=sw_ps[:, :], lhsT=perm_sb[:, :],
                        rhs=src[0:D, 0, s_sl], start=True, stop=True)
                    nc.vector.tensor_mul(
                        out=dst[:, s_sl], in0=sw_ps[:, :], in1=sin_sb[:, s_sl])
                    nc.vector.tensor_add(
                        out=dst[:, s_sl], in0=dst[:, s_sl], in1=tmp[:, s_sl])
            nc.vector.tensor_copy(out=QT_sb[0:D, 0, :], in_=q0r[:, :])
            nc.vector.tensor_copy(out=KT_sb[0:D, 0, :], in_=k0r[:, :])

            # ---- V projection groups (emitted as pipeline filler) ----
            def emit_v_group(st):
                v_ps = psum.tile([P, SC], F32, name="v_ps", tag="s_ps", bufs=3)
                for kt in range(KT8):
                    nc.tensor.matmul(
                        out=v_ps[:, 0:E],
                        lhsT=hT_sb[:, kt, st * P:(st + 1) * P],
                        rhs=wqkvT_sb[:, kt, 2 * E:3 * E],
                        start=(kt == 0), stop=False)
                nc.tensor.matmul(
                    out=v_ps[:, 0:E],
                    lhsT=ones1[0:1, 0:P],
                    rhs=qkvb_sb[0:1, 2 * E:3 * E],
                    start=False, stop=True)
                nc.vector.tensor_copy(out=V_sb[:, st, :], in_=v_ps[:, 0:E])

            # ---- attention: software-pipelined across 8 streams ----
            # stream i = (qc=i//2, pair=i%2).  Per kt slot (2-kt batches to
            # limit PE mode switches) the PE carries: QK(i,kt) row-packed,
            # AV(i,kt-2) col-packed (lag 2 behind exp), denom(qc,kt-4) 4-way
            # (pair1 streams), and outproj groups of qc-1 (pair0 streams).
            # exp alternates ScalarE (table) / VectorE (Schraudolph).
            # 3 PT buffers: 2 dedicated + 1 aliased onto hT_sb (dead after
            # the projections / V groups) -> no PT handoff stall, no extra SBUF
            PT_bufs = [
                consts.tile([P, NKT, 2, SC], BF, name="PTbuf0"),
                consts.tile([P, NKT, 2, SC], BF, name="PTbuf1"),
                hT_sb[:, :, :].rearrange("p a (x y z) -> p (a x) y z",
                                         x=2, y=2, z=SC),
            ]
            PT_by_stream = {}
            OTu_by_stream = {}
            o_ps_by_stream = {}
            d_ps_by_qc = {}
            y_jobs = []  # deferred outproj emitters per qc

            def emit_qk(i, kt):
                qc, pair = i // 2, 1 - i % 2
                q_sl = slice(qc * SC, (qc + 1) * SC)
                k_sl = slice(kt * P, (kt + 1) * P)
                s_ps = psum.tile([P, 2, SC], F32, name="s_ps", tag="s_ps", bufs=3)
                nc.tensor.matmul(
                    out=s_ps[:, 0, :], lhsT=KT_sb[0:D, pair, k_sl],
                    rhs=QT_sb[0:D, pair, q_sl], start=True, stop=True)
                nc.tensor.matmul(
                    out=s_ps[:, 1, :], lhsT=KT_sb[D:P, pair, k_sl],
                    rhs=QT_sb[D:P, pair, q_sl], start=True, stop=True)
                PT = PT_by_stream[i]
                if (kt + i) % 2 == 0:
                    nc.scalar.activation(
                        out=PT[:, kt, :, :], in_=s_ps[:, :, :],
                        func=mybir.ActivationFunctionType.Exp, scale=0.125)
                else:
                    nc.vector.tensor_scalar(
                        out=PT[:, kt, :, :].bitcast(I16), in0=s_ps[:, :, :],
                        scalar1=SCH_A, scalar2=SCH_B,
                        op0=mybir.AluOpType.mult, op1=mybir.AluOpType.add)

            def emit_av(i, kt):
                pair = 1 - i % 2
                PT = PT_by_stream[i]
                o_ps = o_ps_by_stream[i]
                nc.tensor.matmul(
                    out=o_ps[0:D, :],
                    lhsT=V_sb[:, kt, pair * P:pair * P + D],
                    rhs=PT[:, kt, 0, :],
                    start=(kt == 0), stop=(kt == NKT - 1))
                nc.tensor.matmul(
                    out=o_ps[D:P, :],
                    lhsT=V_sb[:, kt, pair * P + D:(pair + 1) * P],
                    rhs=PT[:, kt, 1, :],
                    start=(kt == 0), stop=(kt == NKT - 1))

            def emit_denom(qc, kt):
                d_ps = d_ps_by_qc[qc]
                for j in range(4):
                    nc.tensor.matmul(
                        out=d_ps[32 * j:32 * (j + 1), :],
                        lhsT=ones32[:, :],
                        rhs=PT_by_stream[2 * qc + j // 2][:, kt, j % 2, :],
                        start=(kt == 0), stop=(kt == NKT - 1),
                        tile_position=(0, 32 * j))

            def emit_o_drain(i):
                OTu = work.tile([P, SC], BF, name="OTu", tag="OTu", bufs=2)
                OTu_by_stream[i] = OTu
                if i % 2 == 0:
                    nc.vector.tensor_copy(out=OTu[:, :], in_=o_ps_by_stream[i][:, :])
                else:
                    nc.scalar.copy(out=OTu[:, :], in_=o_ps_by_stream[i][:, :])

            def emit_normalize(qc):
                q_sl = slice(qc * SC, (qc + 1) * SC)
                den_sb = work.tile([P, SC], F32, name="den_sb", tag="den", bufs=2)
                nc.vector.tensor_copy(out=den_sb[:, :], in_=d_ps_by_qc[qc][:, :])
                rcp_f = work.tile([P, SC], F32, name="rcp_f", tag="rcpf", bufs=2)
                nc.vector.reciprocal_approx_fast(out=rcp_f[:, :], in_=den_sb[:, :])
                rcb = work.tile([P, SC], BF, name="rcb", tag="rcb", bufs=2)
                nc.vector.tensor_copy(out=rcb[:, :], in_=rcp_f[:, :])
                for j in range(4):
                    pair, jj = 1 - j // 2, j % 2
                    po = jj * D
                    OTu = OTu_by_stream[2 * qc + j // 2]
                    if qc < NQC - 1:
                        rcp_dr = dram.tile([1, SC], BF, name="rcp_dr",
                                           tag="rcp_dr", bufs=4)
                        nc.sync.dma_start(out=rcp_dr[:, :],
                                          in_=rcb[32 * j:32 * j + 1, :])
                        rbc = work.tile([P, SC], BF, name="rbc", tag="rbc", bufs=4)
                        nc.gpsimd.dma_start(
                            out=rbc[:, :],
                            in_=rcp_dr[0:1, :].to_broadcast([P, SC]))
                        nc.gpsimd.tensor_mul(
                            out=OT_sb[po:po + D, pair, q_sl],
                            in0=OTu[po:po + D, :],
                            in1=rbc[po:po + D, :])
                    else:
                        rt = work.tile([1, SC], BF, name="rt", tag="rt", bufs=4)
                        nc.vector.tensor_copy(out=rt[:, :],
                                              in_=rcb[32 * j:32 * j + 1, :])
                        rbc_ps = psum.tile([P, SC], F32, name="rbc_ps", tag="s_ps", bufs=3)
                        nc.tensor.matmul(
                            out=rbc_ps[:, :], lhsT=ones1[0:1, 0:P],
                            rhs=rt[0:1, :], start=True, stop=True)
                        nc.vector.tensor_mul(
                            out=OT_sb[po:po + D, pair, q_sl],
                            in0=OTu[po:po + D, :],
                            in1=rbc_ps[po:po + D, :])

            def make_outproj_jobs(qc):
                # 8 (sti, oc) groups; emitted two per 2-kt batch in a later
                # stream.  y_sb accumulates the two oc halves before DMA.
                state = {}

                def job(g):
                    sti, oc = g // 2, g % 2
                    st = qc * (SC // P) + sti
                    if oc == 0:
                        state[sti] = work.tile([P, DIM], BF, name="y_sb",
                                               tag="y_sb", bufs=2)
                    y_sb = state[sti]
                    y_ps = psum.tile([P, SC], F32, name="y_ps", tag="s_ps", bufs=3)
                    for et in range(E // P):
                        nc.tensor.matmul(
                            out=y_ps[:, :],
                            lhsT=OT_sb[:, et, st * P:(st + 1) * P],
                            rhs=woutT_sb[:, et, oc * SC:(oc + 1) * SC],
                            start=(et == 0), stop=(et == E // P - 1))
                    if (sti + oc) % 2 == 0:
                        nc.vector.tensor_copy(
                            out=y_sb[:, oc * SC:(oc + 1) * SC], in_=y_ps[:, :])
                    else:
                        nc.scalar.copy(
                            out=y_sb[:, oc * SC:(oc + 1) * SC], in_=y_ps[:, :])
                    if oc == 1:
                        nc.sync.dma_start(
                            out=y_d.ap()[st * P:(st + 1) * P, :], in_=y_sb[:, :])
                return [lambda g=g: job(g) for g in range(8)]

            # V-projection filler queue: stream 0 pops 1/cycle (2 in the last
            # two cycles), stream 1 pops the rest — each group st must land
            # before AV(0, st) / AV(1, st) consumes V_sb[:, st, :].
            vq = list(range(NKT))
            for i in range(8):
                qc, pair = i // 2, i % 2
                PT_by_stream[i] = PT_bufs[i % 3]
                if i >= 1:
                    o_ps_by_stream[i - 1] = psum.tile([P, SC], F32, name="o_ps",
                                                      tag="ob")
                if pair == 1:
                    d_ps_by_qc[qc] = psum.tile([P, SC], F32, name="d_ps", tag="ob")
                jobs = y_jobs.pop(0) if (pair == 1 and y_jobs) else None
                for kt2 in range(NKT // 2):
                    # 2-kt batches per PE mode to limit reconfig drains
                    emit_qk(i, 2 * kt2)
                    emit_qk(i, 2 * kt2 + 1)
                    if i >= 1:
                        emit_av(i - 1, 2 * kt2)
                        emit_av(i - 1, 2 * kt2 + 1)
                    if pair == 1 and kt2 >= 2:
                        emit_denom(qc, 2 * (kt2 - 2))
                        emit_denom(qc, 2 * (kt2 - 2) + 1)
                    if jobs is not None and kt2 >= 4:
                        jobs[2 * (kt2 - 4)]()
                        jobs[2 * (kt2 - 4) + 1]()
                    if i == 0 and vq:
                        emit_v_group(vq.pop(0))
                        if kt2 >= 6:
                            emit_v_group(vq.pop(0))
                    elif i == 1 and vq and kt2 < 6:
                        emit_v_group(vq.pop(0))
                if i >= 1:
                    emit_o_drain(i - 1)
                if pair == 1:
                    for kt in range(NKT - 4, NKT):
                        emit_denom(qc, kt)
                if pair == 0 and qc >= 1:
                    # OTu(2(qc-1)+1) drained above -> normalize previous qc
                    emit_normalize(qc - 1)
                    y_jobs.append(make_outproj_jobs(qc - 1))

            # tail: AV + drain of stream 7, normalize + outproj of qc=3
            o_ps_by_stream[7] = psum.tile([P, SC], F32, name="o_ps", tag="ob")
            for kt in range(NKT):
                emit_av(7, kt)
            emit_o_drain(7)
            emit_normalize(3)
            for jobs in y_jobs:
                for j in jobs:
                    j()
            for j in make_outproj_jobs(3):
                j()

    return nc


def _shard_inputs(hidden_states, cos, sin, qkv_w, qkv_b, out_w):
    """Host-side prep: per-core transposed bf16 shards."""
    hs = np.asarray(hidden_states, dtype=np.float32)
    cos = np.asarray(cos, dtype=np.float32)
    sin = np.asarray(sin, dtype=np.float32)
    qkv_w = np.asarray(qkv_w, dtype=np.float32)
    qkv_b = np.asarray(qkv_b, dtype=np.float32)
    out_w = np.asarray(out_w, dtype=np.float32)

    def bf(x):
        return np.ascontiguousarray(x).astype(_BF_NP)

    hT_b = [bf(hs[b].T) for b in range(B)]
    in_maps = []
    for core in range(NCORES):
        b, g = divmod(core, GROUPS)
        e0 = E * g
        wq = qkv_w[e0:e0 + E]
        wk = qkv_w[H * D + e0:H * D + e0 + E]
        wv = qkv_w[2 * H * D + e0:2 * H * D + e0 + E]
        wqkvT = bf(np.concatenate([wq, wk, wv], axis=0).T)      # [DIM, 768]
        qkvb = bf(np.concatenate([
            qkv_b[e0:e0 + E], qkv_b[H * D + e0:H * D + e0 + E],
            qkv_b[2 * H * D + e0:2 * H * D + e0 + E]])[None, :])  # [1, 768]
        qkvb_col = np.ascontiguousarray(np.concatenate([
            qkv_b[e0:e0 + E], qkv_b[H * D + e0:H * D + e0 + E]]
        )[:, None].astype(np.float32))  # [512, 1] q|k bias as column
        woutT = bf(out_w[:, e0:e0 + E].T)                        # [256, DIM]
        if g == 0:
            c = cos[b].T
            sgn = np.where(np.arange(D) % 2 == 0, -1.0, 1.0)[:, None].astype(np.float32)
            s_ = sin[b].T * sgn
        else:
            c = np.ones((D, S), np.float32)
            s_ = np.zeros((D, S), np.float32)
        perm = np.zeros((D, D), np.float32)
        perm[np.arange(D), np.arange(D) ^ 1] = 1.0
        in_maps.append({
            "hT": hT_b[b],
            "wqkvT": wqkvT,
            "qkvb": qkvb,
            "qkvb_col": qkvb_col,
            "woutT": woutT,
            "cos_t": bf(c),
            "sin_t": bf(s_),
            "perm": bf(perm),
        })
    return in_maps


_last_results = None


def _ensure_axon_hooks():
    """run_bass_kernel_spmd imports antenv.axon_hooks when BASS_TRACE is set;
    this image's antenv lacks that module. Provide a no-op stand-in."""
    try:
        import antenv.axon_hooks  # noqa: F401
    except ImportError:
        import sys as _sys
        import types as _types
        try:
            import antenv
        except ImportError:
            return
        mod = _types.ModuleType("antenv.axon_hooks")
        _state = {"hook": None}
        mod.set_axon_ntff_profile_hook = lambda h: _state.__setitem__("hook", h)
        mod.get_axon_ntff_profile_hook = lambda: _state["hook"]
        _sys.modules["antenv.axon_hooks"] = mod
        antenv.axon_hooks = mod


def kernel(hidden_states, cos, sin, qkv_w, qkv_b, out_w, out_b):
    global _last_results
    _ensure_axon_hooks()
    in_maps = _shard_inputs(hidden_states, cos, sin, qkv_w, qkv_b, out_w)
    nc = _build_nc()
    nc.compile()
    res = run_bass_kernel_spmd(nc, in_maps, core_ids=list(range(NCORES)))
    _last_results = res
    ys = [np.asarray(res.results[c]["y"], dtype=np.float32) for c in range(NCORES)]
    out_b = np.asarray(out_b, dtype=np.float32)
    out = np.stack([
        ys[0] + ys[1] + ys[2] + ys[3] + out_b[None, :],
        ys[4] + ys[5] + ys[6] + ys[7] + out_b[None, :],
    ])
    return out.astype(np.float32)


if __name__ == "__main__":
    nc = _build_nc()
    n_inst = sum(len(bb.instructions) for f in nc.m.functions for bb in f.blocks)
    print(f"built nc with {n_inst} instructions")


# revision 3
# speedup vs baseline: 1.2401x; 1.2401x over previous
"""DiT attention (B=2, S=2048, DIM=1024, H=16, D=64) on 8 TRN2 NeuronCores, v2.

Sharding: data-parallel over B (2) x tensor-parallel over head groups (4);
each core owns (one batch, 4 heads = 2 head-pairs). Host sums the 4 partial
out-projections per batch and adds out_b.

v2 changes vs baseline (259us):
  - PE array packing: QK^T runs 2 heads concurrently via 64x128 row tiling
    (tile_position (0,0)/(64,0)); AV runs 2 heads concurrently via 128x64
    column tiling; softmax denominators via 4-way 128x32 column tiling with
    an all-ones stationary (replaces the padded-V ones-column trick, so AV
    does no wasted columns).
  - softmax exp split across two engines: ScalarE table exp and VectorE
    Schraudolph exp (one tensor_scalar fma -> int16 bits bitcast to bf16;
    ~1.8% rms weight error, zero-mean; output error ~0.1% since attention
    is diffuse).
  - projections restructured for stationary reuse (weights stay loaded for
    4 matmuls) and drained on ScalarE (Identity+bias) to keep DVE free.
  - warmup matmuls during the initial DMA wait so HAM reaches K=8/8 before
    the real work starts; hT DMA'd in half-row chunks so the first
    projection matmuls start at ~1.5us.
"""

import numpy as np
import ml_dtypes

import concourse.bacc as bacc
import concourse.bass as bass
import concourse.mybir as mybir
import concourse.tile as tile
from concourse.bass_utils import run_bass_kernel_spmd

B, S, DIM, H, D = 2, 2048, 1024, 16, 64
NCORES = 8
GROUPS = 4     # head groups (tensor parallel)
HPG = 4        # heads per group -> 2 pairs
E = HPG * D    # 256 e-channels per core per projection
P = 128
SC = 512       # query-chunk width
NKT = S // P   # 16 key tiles
NQC = S // SC  # 4 query chunks
KT8 = DIM // P  # 8 contraction tiles
BF = mybir.dt.bfloat16
F32 = mybir.dt.float32
I16 = mybir.dt.int16

_BF_NP = ml_dtypes.bfloat16

# Schraudolph exp constants for bf16-bit output: bits = round(s*SCH_A + SCH_B)
# approximates exp(0.125*s).  SCH_A = 0.125 * 128/ln2;  C=7.35 zero-means the
# mantissa-linearization error.
SCH_A = float(16.0 / np.log(2.0))
SCH_B = float(127 * 128 - 7.35)


def _build_nc():
    nc = bacc.Bacc(None, target_bir_lowering=False)

    hT_d = nc.declare_dram_parameter("hT", [DIM, S], BF, isOutput=False)
    wqkvT_d = nc.declare_dram_parameter("wqkvT", [DIM, 3 * E], BF, isOutput=False)
    qkvb_d = nc.declare_dram_parameter("qkvb", [1, 3 * E], BF, isOutput=False)
    qkvbc_d = nc.declare_dram_parameter("qkvb_col", [2 * E, 1], F32, isOutput=False)
    woutT_d = nc.declare_dram_parameter("woutT", [E, DIM], BF, isOutput=False)
    cos_d = nc.declare_dram_parameter("cos_t", [D, S], BF, isOutput=False)
    sin_d = nc.declare_dram_parameter("sin_t", [D, S], BF, isOutput=False)
    perm_d = nc.declare_dram_parameter("perm", [D, D], BF, isOutput=False)
    y_d = nc.declare_dram_parameter("y", [S, DIM], BF, isOutput=True)

    hT_t = hT_d.ap().rearrange("(t p) s -> t p s", p=P)        # [8,128,S]
    wqkvT_t = wqkvT_d.ap().rearrange("(t p) e -> t p e", p=P)  # [8,128,768]
    woutT_t = woutT_d.ap().rearrange("(t p) o -> t p o", p=P)  # [2,128,DIM]

    with tile.TileContext(nc) as tc:
        import contextlib
        with contextlib.ExitStack() as ctx:
            consts = ctx.enter_context(tc.tile_pool(name="consts", bufs=1))
            work = ctx.enter_context(tc.tile_pool(name="work", bufs=2))
            dram = ctx.enter_context(tc.tile_pool(name="dram", bufs=2, space="DRAM"))
            psum = ctx.enter_context(tc.tile_pool(name="psum", bufs=2, space="PSUM"))

            # ---- persistent SBUF ----
            hT_sb = consts.tile([P, KT8, S], BF, name="hT_sb")
            wqkvT_sb = consts.tile([P, KT8, 3 * E], BF, name="wqkvT_sb")
            qkvb_sb = consts.tile([1, 3 * E], BF, name="qkvb_sb")
            qkvbc_sb = consts.tile([P, 4, 1], F32, name="qkvbc_sb")
            woutT_sb = consts.tile([P, E // P, DIM], BF, name="woutT_sb")
            cos_sb = consts.tile([D, S], BF, name="cos_sb")
            sin_sb = consts.tile([D, S], BF, name="sin_sb")
            perm_sb = consts.tile([D, D], BF, name="perm_sb")
            ones1 = consts.tile([1, P], BF, name="ones1")
            ones32 = consts.tile([P, 32], BF, name="ones32")
            warm_sb = consts.tile([P, 256], BF, name="warm_sb")
            V_sb = consts.tile([P, NKT, E], BF, name="V_sb")      # pair-packed, no pad
            QT_sb = consts.tile([P, 2, S], BF, name="QT_sb")      # slot=pair, rows 0-63/64-127
            KT_sb = consts.tile([P, 2, S], BF, name="KT_sb")
            OT_sb = consts.tile([P, 2, S], BF, name="OT_sb")
            q0r = consts.tile([D, S], BF, name="q0r")
            k0r = consts.tile([D, S], BF, name="k0r")
            qtmp = consts.tile([D, S], BF, name="qtmp")
            ktmp = consts.tile([D, S], BF, name="ktmp")

            # ---- warmup (keeps HAM busy during initial DMA) ----
            nc.vector.memset(warm_sb[:, :], 0.0)
            warm_ps = psum.tile([P, SC], F32, name="warm_ps", tag="s_ps", bufs=3)
            for i in range(26):
                nc.tensor.matmul(out=warm_ps[:, 0:256], lhsT=warm_sb[:, 0:P],
                                 rhs=warm_sb[:, :], start=True, stop=True)

            # ---- loads: biases first, then per-kt interleaved weight+hT
            # half0 chunks so the first projection groups unlock ASAP ----
            nc.sync.dma_start(out=qkvb_sb[:, :], in_=qkvb_d.ap())
            for t in range(4):
                nc.sync.dma_start(out=qkvbc_sb[:, t, :],
                                  in_=qkvbc_d.ap()[t * P:(t + 1) * P, :])
            h0 = slice(0, 2 * SC)
            h1 = slice(2 * SC, 4 * SC)
            for t in range(KT8):
                nc.sync.dma_start(out=wqkvT_sb[:, t, :], in_=wqkvT_t[t])
                nc.sync.dma_start(out=hT_sb[:, t, h0], in_=hT_t[t][:, h0])
            for t in range(KT8):
                nc.sync.dma_start(out=hT_sb[:, t, h1], in_=hT_t[t][:, h1])
            nc.sync.dma_start(out=cos_sb[:, :], in_=cos_d.ap())
            nc.sync.dma_start(out=sin_sb[:, :], in_=sin_d.ap())
            nc.sync.dma_start(out=perm_sb[:, :], in_=perm_d.ap())
            for t in range(E // P):
                nc.sync.dma_start(out=woutT_sb[:, t, :], in_=woutT_t[t])
            nc.vector.memset(ones1[:, :], 1.0)
            nc.vector.memset(ones32[:, :], 1.0)

            # ---- Q^T / K^T projections: [e, s], stationary w reused x2 ----
            # et=0 groups first so RoPE (which only needs slot 0) overlaps
            # the et=1 matmuls; V projection is deferred into the attention
            # pipeline as filler work.
            def emit_qk_proj(which, et, half):
                dst = QT_sb if which == 0 else KT_sb
                ecols = slice(which * E + et * P, which * E + (et + 1) * P)
                pj = psum.tile([P, 2, SC], F32, name="pj", tag="s_ps", bufs=3)
                for kt in range(KT8):
                    for j in range(2):
                        s_sl = slice((2 * half + j) * SC,
                                     (2 * half + j + 1) * SC)
                        nc.tensor.matmul(
                            out=pj[:, j, :],
                            lhsT=wqkvT_sb[:, kt, ecols],
                            rhs=hT_sb[:, kt, s_sl],
                            start=(kt == 0), stop=(kt == KT8 - 1))
                for j in range(2):
                    s_sl = slice((2 * half + j) * SC,
                                 (2 * half + j + 1) * SC)
                    nc.scalar.activation(
                        out=dst[:, et, s_sl], in_=pj[:, j, :],
                        func=mybir.ActivationFunctionType.Identity,
                        bias=qkvbc_sb[:, which * 2 + et, :])

            # half0 groups (DMA-paced), then half1; et=0 before et=1 inside a
            # half so RoPE's DVE muls can start as early as possible.
            for half in range(2):
                for et in range(2):
                    for which in range(2):
                        emit_qk_proj(which, et, half)
                if half == 0:
                    continue
                # RoPE DVE muls (emitted after et0 of half1... both halves done)
            nc.vector.tensor_mul(out=qtmp[:, :], in0=QT_sb[0:D, 0, :], in1=cos_sb[:, :])
            nc.vector.tensor_mul(out=ktmp[:, :], in0=KT_sb[0:D, 0, :], in1=cos_sb[:, :])

            for src, dst, tmp in ((QT_sb, q0r, qtmp), (KT_sb, k0r, ktmp)):
                for scn in range(NQC):
                    s_sl = slice(scn * SC, (scn + 1) * SC)
                    sw_ps = psum.tile([D, SC], F32, name="sw_ps", tag="s_ps", bufs=3)
                    nc.tensor.matmul(
                        out=sw_ps[:, :], lhsT=perm_sb[:, :],
                        rhs=src[0:D, 0, s_sl], start=True, stop=True)
                    nc.vector.tensor_mul(
                        out=dst[:, s_sl], in0=sw_ps[:, :], in1=sin_sb[:, s_sl])
                    nc.vector.tensor_add(
                        out=dst[:, s_sl], in0=dst[:, s_sl], in1=tmp[:, s_sl])
            nc.vector.tensor_copy(out=QT_sb[0:D, 0, :], in_=q0r[:, :])
            nc.vector.tensor_copy(out=KT_sb[0:D, 0, :], in_=k0r[:, :])

            # ---- V projection groups (emitted as pipeline filler) ----
            def emit_v_group(st):
                v_ps = psum.tile([P, SC], F32, name="v_ps", tag="s_ps", bufs=3)
                for kt in range(KT8):
                    nc.tensor.matmul(
                        out=v_ps[:, 0:E],
                        lhsT=hT_sb[:, kt, st * P:(st + 1) * P],
                        rhs=wqkvT_sb[:, kt, 2 * E:3 * E],
                        start=(kt == 0), stop=False)
                nc.tensor.matmul(
                    out=v_ps[:, 0:E],
                    lhsT=ones1[0:1, 0:P],
                    rhs=qkvb_sb[0:1, 2 * E:3 * E],
                    start=False, stop=True)
                nc.vector.tensor_copy(out=V_sb[:, st, :], in_=v_ps[:, 0:E])

            # ---- attention: software-pipelined across 8 streams ----
            # stream i = (qc=i//2, pair=i%2).  Per kt slot (2-kt batches to
            # limit PE mode switches) the PE carries: QK(i,kt) row-packed,
            # AV(i,kt-2) col-packed (lag 2 behind exp), denom(qc,kt-4) 4-way
            # (pair1 streams), and outproj groups of qc-1 (pair0 streams).
            # exp alternates ScalarE (table) / VectorE (Schraudolph).
            # 3 PT buffers: 2 dedicated + 1 aliased onto hT_sb (dead after
            # the projections / V groups) -> no PT handoff stall, no extra SBUF
            PT_bufs = [
                consts.tile([P, NKT, 2, SC], BF, name="PTbuf0"),
                consts.tile([P, NKT, 2, SC], BF, name="PTbuf1"),
                hT_sb[:, :, :].rearrange("p a (x y z) -> p (a x) y z",
                                         x=2, y=2, z=SC),
            ]
            PT_by_stream = {}
            OTu_by_stream = {}
            o_ps_by_stream = {}
            d_ps_by_qc = {}
            y_jobs = []  # deferred outproj emitters per qc

            def emit_qk(i, kt):
                qc, pair = i // 2, 1 - i % 2
                q_sl = slice(qc * SC, (qc + 1) * SC)
                k_sl = slice(kt * P, (kt + 1) * P)
                s_ps = psum.tile([P, 2, SC], F32, name="s_ps", tag="s_ps", bufs=3)
                nc.tensor.matmul(
                    out=s_ps[:, 0, :], lhsT=KT_sb[0:D, pair, k_sl],
                    rhs=QT_sb[0:D, pair, q_sl], start=True, stop=True)
                nc.tensor.matmul(
                    out=s_ps[:, 1, :], lhsT=KT_sb[D:P, pair, k_sl],
                    rhs=QT_sb[D:P, pair, q_sl], start=True, stop=True)
                PT = PT_by_stream[i]
                if (kt + i) % 2 == 0:
                    nc.scalar.activation(
                        out=PT[:, kt, :, :], in_=s_ps[:, :, :],
                        func=mybir.ActivationFunctionType.Exp, scale=0.125)
                else:
                    nc.vector.tensor_scalar(
                        out=PT[:, kt, :, :].bitcast(I16), in0=s_ps[:, :, :],
                        scalar1=SCH_A, scalar2=SCH_B,
                        op0=mybir.AluOpType.mult, op1=mybir.AluOpType.add)

            def emit_av(i, kt):
                pair = 1 - i % 2
                PT = PT_by_stream[i]
                o_ps = o_ps_by_stream[i]
                nc.tensor.matmul(
                    out=o_ps[0:D, :],
                    lhsT=V_sb[:, kt, pair * P:pair * P + D],
                    rhs=PT[:, kt, 0, :],
                    start=(kt == 0), stop=(kt == NKT - 1))
                nc.tensor.matmul(
                    out=o_ps[D:P, :],
                    lhsT=V_sb[:, kt, pair * P + D:(pair + 1) * P],
                    rhs=PT[:, kt, 1, :],
                    start=(kt == 0), stop=(kt == NKT - 1))

            def emit_denom(qc, kt):
                d_ps = d_ps_by_qc[qc]
                for j in range(4):
                    nc.tensor.matmul(
                        out=d_ps[32 * j:32 * (j + 1), :],
                        lhsT=ones32[:, :],
                        rhs=PT_by_stream[2 * qc + j // 2][:, kt, j % 2, :],
                        start=(kt == 0), stop=(kt == NKT - 1),
                        tile_position=(0, 32 * j))

            def emit_o_drain(i):
                OTu = work.tile([P, SC], BF, name="OTu", tag="OTu", bufs=2)
                OTu_by_stream[i] = OTu
                if i % 2 == 0:
                    nc.vector.tensor_copy(out=OTu[:, :], in_=o_ps_by_stream[i][:, :])
                else:
                    nc.scalar.copy(out=OTu[:, :], in_=o_ps_by_stream[i][:, :])

            def emit_normalize(qc):
                q_sl = slice(qc * SC, (qc + 1) * SC)
                den_sb = work.tile([P, SC], F32, name="den_sb", tag="den", bufs=2)
                nc.scalar.copy(out=den_sb[:, :], in_=d_ps_by_qc[qc][:, :])
                rcp_f = work.tile([P, SC], F32, name="rcp_f", tag="rcpf", bufs=2)
                nc.vector.reciprocal_approx_fast(out=rcp_f[:, :], in_=den_sb[:, :])
                rcb = work.tile([P, SC], BF, name="rcb", tag="rcb", bufs=2)
                nc.scalar.copy(out=rcb[:, :], in_=rcp_f[:, :])
                for j in range(4):
                    pair, jj = 1 - j // 2, j % 2
                    po = jj * D
                    OTu = OTu_by_stream[2 * qc + j // 2]
                    if qc < NQC - 1:
                        rcp_dr = dram.tile([1, SC], BF, name="rcp_dr",
                                           tag="rcp_dr", bufs=4)
                        nc.sync.dma_start(out=rcp_dr[:, :],
                                          in_=rcb[32 * j:32 * j + 1, :])
                        rbc = work.tile([P, SC], BF, name="rbc", tag="rbc", bufs=4)
                        nc.gpsimd.dma_start(
                            out=rbc[:, :],
                            in_=rcp_dr[0:1, :].to_broadcast([P, SC]))
                        nc.gpsimd.tensor_mul(
                            out=OT_sb[po:po + D, pair, q_sl],
                            in0=OTu[po:po + D, :],
                            in1=rbc[po:po + D, :])
                    else:
                        rt = work.tile([1, SC], BF, name="rt", tag="rt", bufs=4)
                        nc.vector.tensor_copy(out=rt[:, :],
                                              in_=rcb[32 * j:32 * j + 1, :])
                        rbc_ps = psum.tile([P, SC], F32, name="rbc_ps", tag="s_ps", bufs=3)
                        nc.tensor.matmul(
                            out=rbc_ps[:, :], lhsT=ones1[0:1, 0:P],
                            rhs=rt[0:1, :], start=True, stop=True)
                        nc.vector.tensor_mul(
                            out=OT_sb[po:po + D, pair, q_sl],
                            in0=OTu[po:po + D, :],
                            in1=rbc_ps[po:po + D, :])

            def make_outproj_jobs(qc):
                # 8 (sti, oc) groups; emitted two per 2-kt batch in a later
                # stream.  y_sb accumulates the two oc halves before DMA.
                state = {}

                def job(g):
                    sti, oc = g // 2, g % 2
                    st = qc * (SC // P) + sti
                    if oc == 0:
                        state[sti] = work.tile([P, DIM], BF, name="y_sb",
                                               tag="y_sb", bufs=2)
                    y_sb = state[sti]
                    y_ps = psum.tile([P, SC], F32, name="y_ps", tag="s_ps", bufs=3)
                    for et in range(E // P):
                        nc.tensor.matmul(
                            out=y_ps[:, :],
                            lhsT=OT_sb[:, et, st * P:(st + 1) * P],
                            rhs=woutT_sb[:, et, oc * SC:(oc + 1) * SC],
                            start=(et == 0), stop=(et == E // P - 1))
                    if (sti + oc) % 2 == 0:
                        nc.vector.tensor_copy(
                            out=y_sb[:, oc * SC:(oc + 1) * SC], in_=y_ps[:, :])
                    else:
                        nc.scalar.copy(
                            out=y_sb[:, oc * SC:(oc + 1) * SC], in_=y_ps[:, :])
                    if oc == 1:
                        nc.sync.dma_start(
                            out=y_d.ap()[st * P:(st + 1) * P, :], in_=y_sb[:, :])
                return [lambda g=g: job(g) for g in range(8)]

            # V-projection filler queue: stream 0 pops 1/cycle (2 in the last
            # two cycles), stream 1 pops the rest — each group st must land
            # before AV(0, st) / AV(1, st) consumes V_sb[:, st, :].
            vq = list(range(NKT))
            for i in range(8):
                qc, pair = i // 2, i % 2
                PT_by_stream[i] = PT_bufs[i % 3]
                if i >= 1:
                    o_ps_by_stream[i - 1] = psum.tile([P, SC], F32, name="o_ps",
                                                      tag="ob")
                if pair == 1:
                    d_ps_by_qc[qc] = psum.tile([P, SC], F32, name="d_ps", tag="ob")
                jobs = y_jobs.pop(0) if (pair == 1 and y_jobs) else None
                for kt2 in range(NKT // 2):
                    # 2-kt batches per PE mode to limit reconfig drains
                    emit_qk(i, 2 * kt2)
                    emit_qk(i, 2 * kt2 + 1)
                    if i >= 1:
                        emit_av(i - 1, 2 * kt2)
                        emit_av(i - 1, 2 * kt2 + 1)
                    if pair == 1 and kt2 >= 2:
                        emit_denom(qc, 2 * (kt2 - 2))
                        emit_denom(qc, 2 * (kt2 - 2) + 1)
                    if jobs is not None and kt2 >= 4:
                        jobs[2 * (kt2 - 4)]()
                        jobs[2 * (kt2 - 4) + 1]()
                    if i == 0 and vq:
                        emit_v_group(vq.pop(0))
                        if kt2 >= 6:
                            emit_v_group(vq.pop(0))
                    elif i == 1 and vq and kt2 < 6:
                        emit_v_group(vq.pop(0))
                if i >= 1:
                    emit_o_drain(i - 1)
                if pair == 1:
                    for kt in range(NKT - 4, NKT):
                        emit_denom(qc, kt)
                if pair == 0 and qc >= 1:
                    # OTu(2(qc-1)+1) drained above -> normalize previous qc
                    emit_normalize(qc - 1)
                    y_jobs.append(make_outproj_jobs(qc - 1))

            # tail: AV + drain of stream 7, normalize + outproj of qc=3
            o_ps_by_stream[7] = psum.tile([P, SC], F32, name="o_ps", tag="ob")
            for kt in range(NKT):
                emit_av(7, kt)
            emit_o_drain(7)
            emit_normalize(3)
            for jobs in y_jobs:
                for j in jobs:
                    j()
            for j in make_outproj_jobs(3):
                j()

    return nc


def _shard_inputs(hidden_states, cos, sin, qkv_w, qkv_b, out_w):
    """Host-side prep: per-core transposed bf16 shards."""
    hs = np.asarray(hidden_states, dtype=np.float32)
    cos = np.asarray(cos, dtype=np.float32)
    sin = np.asarray(sin, dtype=np.float32)
    qkv_w = np.asarray(qkv_w, dtype=np.float32)
    qkv_b = np.asarray(qkv_b, dtype=np.float32)
    out_w = np.asarray(out_w, dtype=np.float32)

    def bf(x):
        return np.ascontiguousarray(x).astype(_BF_NP)

    hT_b = [bf(hs[b].T) for b in range(B)]
    in_maps = []
    for core in range(NCORES):
        b, g = divmod(core, GROUPS)
        e0 = E * g
        wq = qkv_w[e0:e0 + E]
        wk = qkv_w[H * D + e0:H * D + e0 + E]
        wv = qkv_w[2 * H * D + e0:2 * H * D + e0 + E]
        wqkvT = bf(np.concatenate([wq, wk, wv], axis=0).T)      # [DIM, 768]
        qkvb = bf(np.concatenate([
            qkv_b[e0:e0 + E], qkv_b[H * D + e0:H * D + e0 + E],
            qkv_b[2 * H * D + e0:2 * H * D + e0 + E]])[None, :])  # [1, 768]
        qkvb_col = np.ascontiguousarray(np.concatenate([
            qkv_b[e0:e0 + E], qkv_b[H * D + e0:H * D + e0 + E]]
        )[:, None].astype(np.float32))  # [512, 1] q|k bias as column
        woutT = bf(out_w[:, e0:e0 + E].T)                        # [256, DIM]
        if g == 0:
            c = cos[b].T
            sgn = np.where(np.arange(D) % 2 == 0, -1.0, 1.0)[:, None].astype(np.float32)
            s_ = sin[b].T * sgn
        else:
            c = np.ones((D, S), np.float32)
            s_ = np.zeros((D, S), np.float32)
        perm = np.zeros((D, D), np.float32)
        perm[np.arange(D), np.arange(D) ^ 1] = 1.0
        in_maps.append({
            "hT": hT_b[b],
            "wqkvT": wqkvT,
            "qkvb": qkvb,
            "qkvb_col": qkvb_col,
            "woutT": woutT,
            "cos_t": bf(c),
            "sin_t": bf(s_),
            "perm": bf(perm),
        })
    return in_maps


_last_results = None


def _ensure_axon_hooks():
    """run_bass_kernel_spmd imports antenv.axon_hooks when BASS_TRACE is set;
    this image's antenv lacks that module. Provide a no-op stand-in."""
    try:
        import antenv.axon_hooks  # noqa: F401
    except ImportError:
        import sys as _sys
        import types as _types
        try:
            import antenv
        except ImportError:
            return
        mod = _types.ModuleType("antenv.axon_hooks")
        _state = {"hook": None}
        mod.set_axon_ntff_profile_hook = lambda h: _state.__setitem__("hook", h)
        mod.get_axon_ntff_profile_hook = lambda: _state["hook"]
        _sys.modules["antenv.axon_hooks"] = mod
        antenv.axon_hooks = mod


def kernel(hidden_states, cos, sin, qkv_w, qkv_b, out_w, out_b):
    global _last_results
    _ensure_axon_hooks()
    in_maps = _shard_inputs(hidden_states, cos, sin, qkv_w, qkv_b, out_w)
    nc = _build_nc()
    nc.compile()
    res = run_bass_kernel_spmd(nc, in_maps, core_ids=list(range(NCORES)))
    _last_results = res
    ys = [np.asarray(res.results[c]["y"], dtype=np.float32) for c in range(NCORES)]
    out_b = np.asarray(out_b, dtype=np.float32)
    out = np.stack([
        ys[0] + ys[1] + ys[2] + ys[3] + out_b[None, :],
        ys[4] + ys[5] + ys[6] + ys[7] + out_b[None, :],
    ])
    return out.astype(np.float32)


if __name__ == "__main__":
    nc = _build_nc()
    n_inst = sum(len(bb.instructions) for f in nc.m.functions for bb in f.blocks)
    print(f"built nc with {n_inst} instructions")
